# revision 8
# baseline (speedup 1.0000x reference)
"""Trainium2 Bass kernel for nn_Net_34136400069284 (topk_masking).

Computes: conv2d(x, kernel, VALID) -> (B,1024,27,27); for t = -out+bias and
t = +out+bias: per-pixel 257th-smallest along channels (exact order stat),
soft mask sigmoid(thr - t), 6x6/stride-3 avg-pool, 2x2 block mean ->
(B,1024,4,4) per branch.

Strategy (per core, data-parallel over batch, 16 samples/core):
 - im2col patches [109,729] per sample (108 taps + ones row for bias fold)
 - conv as PE matmul vs augmented [109,2048] weights ([-K|+K] with bias row)
   -> per pixel-chunk [128,2048] activations T (both branches side by side)
 - per-pixel kth-smallest by bracketed counting: Gaussian-moment init
   (mean via matmul column, sigma via ||patch||^2 matmul + inv-sqrt bit trick)
   then 9 count passes (1 probe + 8 Illinois regula-falsi) using fused
   tensor_scalar(is_le)+accum; exact finisher max{t <= hi} via
   tensor_tensor_reduce
 - mask = sigmoid(thr - t) on ScalarE (bf16), both pool stages fused into one
   PE matmul vs precomputed integer weights, scaled 1/144 on PSUM evict
 - PE transpose -> (1024,4,4) fp32 outputs
"""
import sys
sys.path.insert(0, "/opt/trn_rl_repo")
import math
import numpy as np

N_CORES = 8
BPC = 16           # batch per core
CH = 1024
KDIM = 108
HW = 729
NCHUNK = 6
TAU = 256.5
Z0 = -0.6744897501960817
DELTA = 0.30
N_ITER = 9         # 1 init probe + 8 Illinois regula-falsi passes
SHIFT = 128.0   # makes all activations positive (|t| << 128) so the
                # masked-max finisher can use is_le*t; cancels in sigmoid
SQRT_MAGIC = 0x1FBD1DF5

_cache = {}


def _phi(z):
    return 0.5 * (1.0 + math.erf(z / math.sqrt(2.0)))


def _host_consts(kernel_w, bias):
    kmat = np.ascontiguousarray(kernel_w.reshape(CH, KDIM)).astype(np.float32)
    b = np.ascontiguousarray(bias.reshape(CH)).astype(np.float32)
    kt = np.zeros((KDIM + 1, 2 * CH), np.float32)
    kt[1:, :CH] = -kmat.T
    kt[1:, CH:] = kmat.T
    kt[0, :CH] = b + SHIFT
    kt[0, CH:] = b + SHIFT
    kbar = kmat.mean(axis=0)
    mb = float(b.mean())
    siderhs = np.zeros((KDIM + 1, 2), np.float32)
    siderhs[1:, 0] = -kbar
    siderhs[1:, 1] = kbar
    siderhs[0, 0] = mb + SHIFT
    siderhs[0, 1] = mb + SHIFT
    alpha = float((kmat.astype(np.float64) ** 2).sum(axis=1).mean() / KDIM)
    bvar = float(b.astype(np.float64).var())
    a_sig = alpha
    b_sig = bvar - alpha   # sig2 = a_sig * (||p||^2 + 1) + b_sig

    # pooling matrix: integer window-multiplicity weights, scaled 1/144 later
    a_int = np.zeros((16, HW), np.float32)
    for fi in range(4):
        for fj in range(4):
            f = fi * 4 + fj
            for wi in (2 * fi, 2 * fi + 1):
                for wj in (2 * fj, 2 * fj + 1):
                    for di in range(6):
                        for dj in range(6):
                            p = (3 * wi + di) * 27 + (3 * wj + dj)
                            a_int[f, p] += 1.0
    at = np.zeros((NCHUNK * 128, 16), np.float32)
    at[:HW] = a_int.T
    at_bf16_bits = (at.view(np.uint32) >> 16).astype(np.uint16)  # exact ints
    return kt, siderhs, a_sig, b_sig, at_bf16_bits


def _build_program(n_samples, a_sig, b_sig, debug=False):
    from concourse import bacc, mybir
    import concourse.bass as bass
    from concourse.tile import TileContext
    from concourse.masks import make_identity

    f32 = mybir.dt.float32
    bf16 = mybir.dt.bfloat16
    i32 = mybir.dt.int32
    Alu = mybir.AluOpType
    Act = mybir.ActivationFunctionType

    c_lo0 = float(CH * _phi(Z0 - DELTA))
    c_hi0 = float(CH * _phi(Z0 + DELTA))

    nc = bacc.Bacc("TRN2", target_bir_lowering=False, debug=False)
    Xh = nc.dram_tensor("x", [n_samples, 3, 32, 32], f32, kind="ExternalInput")
    KTh = nc.dram_tensor("kt", [KDIM + 1, 2 * CH], f32, kind="ExternalInput")
    SRh = nc.dram_tensor("siderhs", [KDIM + 1, 2], f32, kind="ExternalInput")
    ATh = nc.dram_tensor("at", [NCHUNK * 128, 16], bf16, kind="ExternalInput")
    O1h = nc.dram_tensor("o1", [n_samples, CH, 16], f32, kind="ExternalOutput")
    O2h = nc.dram_tensor("o2", [n_samples, CH, 16], f32, kind="ExternalOutput")
    if debug:
        DTHRh = nc.dram_tensor("dthr", [n_samples, 128, 12], f32,
                               kind="ExternalOutput")
        DM0h = nc.dram_tensor("dm0", [n_samples, 128, 12], f32,
                              kind="ExternalOutput")
        DTh = nc.dram_tensor("dt0", [128, 2048], f32, kind="ExternalOutput")

    from contextlib import ExitStack
    with TileContext(nc) as tc:
        with ExitStack() as ctx:
            cpool = ctx.enter_context(tc.tile_pool(name="const", bufs=1))
            ppool = ctx.enter_context(tc.tile_pool(name="patch", bufs=3))
            p2pool = ctx.enter_context(tc.tile_pool(name="patch2", bufs=2))
            tpool = ctx.enter_context(tc.tile_pool(name="tdata", bufs=8))
            jpool = ctx.enter_context(tc.tile_pool(name="junk", bufs=2))
            mpool = ctx.enter_context(tc.tile_pool(name="mask", bufs=3))
            spool = ctx.enter_context(tc.tile_pool(name="state", bufs=2))
            opool = ctx.enter_context(tc.tile_pool(name="outsb", bufs=2))
            psc = ctx.enter_context(tc.tile_pool(name="psc", bufs=1, space="PSUM"))
            psp = ctx.enter_context(tc.tile_pool(name="psp", bufs=1, space="PSUM"))
            psm = ctx.enter_context(tc.tile_pool(name="psm", bufs=1, space="PSUM"))
            # ---- constants ----
            kts = cpool.tile([KDIM + 1, 2 * CH], f32)
            nc.sync.dma_start(kts[:], KTh.ap())
            srs = cpool.tile([KDIM + 1, 2], f32)
            nc.sync.dma_start(srs[:], SRh.ap())
            ones_r = cpool.tile([KDIM + 1, 1], f32)
            nc.vector.memset(ones_r[:], 1.0)
            ats = cpool.tile([128, NCHUNK * 16], bf16)
            for c in range(NCHUNK):
                nc.sync.dma_start(ats[:, 16 * c:16 * c + 16],
                                  ATh.ap()[128 * c:128 * c + 128, :])
            ident = cpool.tile([16, 16], f32)
            make_identity(nc, ident[:])

            for s in range(n_samples):
                # ---- im2col ----
                pt = ppool.tile([KDIM + 1, NCHUNK * 128], f32, tag="pt")
                nc.gpsimd.memset(pt[:, HW:], 0.0)
                nc.vector.memset(pt[0:1, 0:HW], 1.0)
                for ci in range(3):
                    for kh in range(6):
                        k0 = 1 + ci * 36 + kh * 6
                        src = bass.AP(Xh, s * 3072 + ci * 1024 + kh * 32,
                                      [[1, 6], [32, 27], [1, 27]])
                        dst = pt[k0:k0 + 6, 0:HW].rearrange(
                            "p (a b) -> p a b", a=27)
                        nc.sync.dma_start(dst, src)
                pt2 = p2pool.tile([KDIM + 1, NCHUNK * 128], f32, tag="pt2")
                nc.vector.tensor_mul(pt2[:, 0:HW], pt[:, 0:HW], pt[:, 0:HW])
                nc.gpsimd.memset(pt2[:, HW:], 0.0)

                # ---- conv + side matmuls, evict to SBUF ----
                tsb = []
                side = spool.tile([128, 4 * NCHUNK], f32, tag="side")
                for c in range(NCHUNK):
                    t_c = tpool.tile([128, 2 * CH], f32, tag="T")
                    lhs = pt[:, 128 * c:128 * c + 128]
                    for half in range(2):
                        ps = psc.tile([128, CH], f32, tag="cps")
                        for j in range(2):
                            nc.tensor.matmul(
                                ps[:, 512 * j:512 * j + 512], lhs,
                                kts[:, CH * half + 512 * j:
                                    CH * half + 512 * j + 512],
                                start=True, stop=True)
                        nc.scalar.activation(t_c[:, CH * half:CH * half + CH],
                                             ps[:], Act.Copy)
                    sps = psm.tile([128, 3], f32, tag="sps")
                    nc.tensor.matmul(sps[:, 0:2], lhs, srs[:],
                                     start=True, stop=True)
                    nc.tensor.matmul(sps[:, 2:3],
                                     pt2[:, 128 * c:128 * c + 128],
                                     ones_r[:], start=True, stop=True)
                    nc.scalar.activation(side[:, 4 * c:4 * c + 3], sps[:],
                                         Act.Copy)
                    tsb.append(t_c)

                # ---- state init ----
                sa3 = side[:].rearrange("p (c q) -> p c q", q=4)
                pn_v = sa3[:, :, 2:3].rearrange("p c q -> p (c q)")  # [128,6]
                sg2 = spool.tile([128, NCHUNK], f32, tag="sg2")
                # sig2 = a_sig * pn2p1 + b_sig  (per-chunk cols)
                nc.vector.tensor_scalar(sg2[:], pn_v, a_sig, b_sig,
                                        Alu.mult, Alu.add)
                # inv-sqrt bit trick + 1 Newton iteration -> sigma
                xb = spool.tile([128, NCHUNK], i32, tag="xb")
                nc.vector.tensor_scalar(xb[:], sg2[:].bitcast(i32), 1,
                                        None, Alu.arith_shift_right)
                nc.vector.tensor_scalar(xb[:], xb[:], SQRT_MAGIC, None,
                                        Alu.add)
                x0 = xb[:].bitcast(f32)
                rc = spool.tile([128, NCHUNK], f32, tag="rc")
                nc.vector.reciprocal(rc[:], x0)
                tn = spool.tile([128, NCHUNK], f32, tag="tn")
                nc.vector.tensor_mul(tn[:], sg2[:], rc[:])
                nc.vector.tensor_add(tn[:], tn[:], x0)
                sig = spool.tile([128, NCHUNK], f32, tag="sig")
                nc.vector.tensor_scalar_mul(sig[:], tn[:], 0.5)

                st_m = spool.tile([128, 12], f32, tag="st_m")
                st_lo = spool.tile([128, 12], f32, tag="st_lo")
                st_hi = spool.tile([128, 12], f32, tag="st_hi")
                st_clo = spool.tile([128, 12], f32, tag="st_clo")
                st_chi = spool.tile([128, 12], f32, tag="st_chi")
                st_ge = spool.tile([128, 12], f32, tag="st_ge")
                st_lge = spool.tile([128, 12], f32, tag="st_lge")
                st_geu = spool.tile([128, 12], mybir.dt.uint32, tag="st_geu")
                st_ngeu = spool.tile([128, 12], mybir.dt.uint32, tag="st_ngeu")
                st_cnt = spool.tile([128, 12], f32, tag="st_cnt")
                st_thr = spool.tile([128, 12], f32, tag="st_thr")
                sA = spool.tile([128, 12], f32, tag="sA")
                sB = spool.tile([128, 12], f32, tag="sB")
                sC = spool.tile([128, 12], f32, tag="sC")

                sig_b = sig[:].unsqueeze(2).to_broadcast([128, NCHUNK, 2])
                mu_v = sa3[:, :, 0:2]
                m_v = st_m[:].rearrange("p (c h) -> p c h", h=2)
                lo_v = st_lo[:].rearrange("p (c h) -> p c h", h=2)
                hi_v = st_hi[:].rearrange("p (c h) -> p c h", h=2)
                # m0 = mu + Z0*sigma ; lo/hi = m0 -/+ DELTA*sigma
                nc.vector.scalar_tensor_tensor(m_v, sig_b, Z0, mu_v,
                                               op0=Alu.mult, op1=Alu.add)
                nc.vector.scalar_tensor_tensor(lo_v, sig_b, -DELTA, m_v,
                                               op0=Alu.mult, op1=Alu.add)
                nc.vector.scalar_tensor_tensor(hi_v, sig_b, DELTA, m_v,
                                               op0=Alu.mult, op1=Alu.add)
                nc.vector.memset(st_clo[:], c_lo0)
                nc.vector.memset(st_chi[:], c_hi0)
                if debug:
                    nc.sync.dma_start(DM0h.ap()[s], st_m[:])

                # ---- iterations ----
                for it in range(N_ITER):
                    for c in range(NCHUNK):
                        for half in range(2):
                            col = 2 * c + half
                            junk = jpool.tile([128, CH], f32, tag="junk")
                            nc.vector.tensor_scalar(
                                junk[:], tsb[c][:, CH * half:CH * half + CH],
                                st_m[:, col:col + 1], 0.0,
                                Alu.is_le, Alu.add,
                                accum_out=st_cnt[:, col:col + 1])
                    nc.vector.tensor_scalar(st_ge[:], st_cnt[:], TAU, None,
                                            Alu.is_ge)
                    nc.vector.tensor_scalar(st_geu[:], st_cnt[:], TAU, None,
                                            Alu.is_ge)
                    nc.vector.tensor_scalar(st_ngeu[:], st_cnt[:], TAU, None,
                                            Alu.is_lt)
                    if it > 0:
                        # Illinois: stale-side count pulled toward TAU
                        nc.vector.tensor_tensor(sA[:], st_ge[:], st_lge[:],
                                                op=Alu.is_equal)
                        nc.vector.tensor_mul(sB[:], sA[:], st_ge[:])  # both ge
                        nc.vector.tensor_sub(sA[:], sA[:], sB[:])     # both lt
                        nc.vector.scalar_tensor_tensor(sC[:], st_clo[:], TAU,
                                                       sB[:], op0=Alu.subtract,
                                                       op1=Alu.mult)
                        nc.vector.scalar_tensor_tensor(st_clo[:], sC[:], -0.5,
                                                       st_clo[:], op0=Alu.mult,
                                                       op1=Alu.add)
                        nc.vector.scalar_tensor_tensor(sC[:], st_chi[:], TAU,
                                                       sA[:], op0=Alu.subtract,
                                                       op1=Alu.mult)
                        nc.vector.scalar_tensor_tensor(st_chi[:], sC[:], -0.5,
                                                       st_chi[:], op0=Alu.mult,
                                                       op1=Alu.add)
                    nc.vector.copy_predicated(st_hi[:], st_geu[:], st_m[:])
                    nc.vector.copy_predicated(st_chi[:], st_geu[:], st_cnt[:])
                    nc.vector.copy_predicated(st_lo[:], st_ngeu[:], st_m[:])
                    nc.vector.copy_predicated(st_clo[:], st_ngeu[:], st_cnt[:])
                    st_ge, st_lge = st_lge, st_ge
                    if it < N_ITER - 1:
                        # regula falsi: m = lo - (c_lo-TAU)*(hi-lo)/(c_hi-c_lo)
                        nc.vector.tensor_sub(sA[:], st_hi[:], st_lo[:])
                        nc.vector.tensor_sub(sB[:], st_chi[:], st_clo[:])
                        nc.vector.reciprocal(sB[:], sB[:])
                        nc.vector.scalar_tensor_tensor(sC[:], st_clo[:], TAU,
                                                       sA[:], op0=Alu.subtract,
                                                       op1=Alu.mult)
                        nc.vector.tensor_mul(sC[:], sC[:], sB[:])
                        nc.vector.tensor_sub(st_m[:], st_lo[:], sC[:])

                # ---- exact finisher + sigmoid + pooling ----
                pooled = psp.tile([16, 2 * CH], f32, tag="pool")
                for c in range(NCHUNK):
                    msk = mpool.tile([128, 2 * CH], bf16, tag="msk")
                    for half in range(2):
                        col = 2 * c + half
                        t_half = tsb[c][:, CH * half:CH * half + CH]
                        junk = jpool.tile([128, CH], f32, tag="junk")
                        nc.vector.scalar_tensor_tensor(
                            junk[:], t_half, st_hi[:, col:col + 1], t_half,
                            op0=Alu.is_le, op1=Alu.mult)
                        nc.vector.tensor_reduce(st_thr[:, col:col + 1],
                                                junk[:],
                                                axis=mybir.AxisListType.X,
                                                op=Alu.max)
                        nc.scalar.activation(msk[:, CH * half:CH * half + CH],
                                             t_half, Act.Sigmoid,
                                             bias=st_thr[:, col:col + 1],
                                             scale=-1.0)
                    for j in range(4):
                        nc.tensor.matmul(pooled[:, 512 * j:512 * j + 512],
                                         ats[:, 16 * c:16 * c + 16],
                                         msk[:, 512 * j:512 * j + 512],
                                         start=(c == 0), stop=(c == NCHUNK - 1))
                if debug:
                    nc.sync.dma_start(DTHRh.ap()[s], st_thr[:])
                    if s == 0:
                        nc.sync.dma_start(DTh.ap()[:], tsb[0][:])

                # ---- scale, transpose, store ----
                pooled_sb = opool.tile([16, 2 * CH], f32, tag="pooled_sb")
                nc.scalar.activation(pooled_sb[:], pooled[:], Act.Copy,
                                     scale=1.0 / 144.0)
                pst = psm.tile([128, 256], f32, tag="pst")
                for j in range(16):
                    nc.tensor.transpose(pst[:, 16 * j:16 * j + 16],
                                        pooled_sb[:, 128 * j:128 * j + 128],
                                        ident[:])
                outt = opool.tile([128, 256], f32, tag="outt")
                nc.scalar.activation(outt[:], pst[:], Act.Copy)
                for br, Oh in ((0, O1h), (1, O2h)):
                    src = outt[:, 128 * br:128 * br + 128].rearrange(
                        "p (j f) -> p j f", f=16)
                    dst = Oh.ap()[s].rearrange("(j r) f -> r j f", r=128)
                    nc.sync.dma_start(dst, src)
    nc.compile()
    return nc


def _get_program(n_samples, a_sig, b_sig, debug=False):
    key = (n_samples, round(a_sig, 9), round(b_sig, 9), debug)
    if key not in _cache:
        _cache[key] = _build_program(n_samples, a_sig, b_sig, debug)
    return _cache[key]


def _run(x, kernel_w, bias, n_samples_per_core=BPC, debug=False, trace=False):
    from concourse.bass_utils import run_bass_kernel_spmd
    import ml_dtypes

    kt, siderhs, a_sig, b_sig, at_bits = _host_consts(kernel_w, bias)
    at_bf = at_bits.view(ml_dtypes.bfloat16)
    nc = _get_program(n_samples_per_core, a_sig, b_sig, debug)
    n_cores = x.shape[0] // n_samples_per_core
    in_maps = []
    for i in range(n_cores):
        xs = np.ascontiguousarray(
            x[i * n_samples_per_core:(i + 1) * n_samples_per_core]
        ).astype(np.float32)
        in_maps.append({"x": xs, "kt": kt, "siderhs": siderhs,
                       "at": at_bf})
    res = run_bass_kernel_spmd(nc, in_maps, list(range(n_cores)), trace=trace)
    o1 = np.concatenate([r["o1"] for r in res.results], axis=0)
    o2 = np.concatenate([r["o2"] for r in res.results], axis=0)
    B = o1.shape[0]
    out1 = o1.reshape(B, CH, 4, 4)
    out2 = o2.reshape(B, CH, 4, 4)
    return (out1, out2), res


def kernel(x, kernel, bias):
    (out1, out2), _ = _run(np.asarray(x), np.asarray(kernel),
                           np.asarray(bias))
    return out1, out2


# revision 13
# speedup vs baseline: 1.1752x; 1.1752x over previous
"""Trainium2 Bass kernel for nn_Net_34136400069284 (topk_masking).

Computes: conv2d(x, kernel, VALID) -> (B,1024,27,27); for t = -out+bias and
t = +out+bias: per-pixel 257th-smallest along channels (exact order stat),
soft mask sigmoid(thr - t), 6x6/stride-3 avg-pool, 2x2 block mean ->
(B,1024,4,4) per branch.

Strategy (per core, data-parallel over batch, 16 samples/core):
 - im2col patches [109,729] per sample (108 taps + ones row for bias fold)
 - conv as PE matmul vs augmented [109,2048] weights ([-K|+K] with bias row)
   -> per pixel-chunk [128,2048] activations T (both branches side by side)
 - per-pixel kth-smallest by bracketed counting: Gaussian-moment init
   (mean via matmul column, sigma via ||patch||^2 matmul + inv-sqrt bit trick)
   then 9 count passes (1 probe + 8 Illinois regula-falsi) using fused
   tensor_scalar(is_le)+accum; exact finisher max{t <= hi} via
   tensor_tensor_reduce
 - mask = sigmoid(thr - t) on ScalarE (bf16), both pool stages fused into one
   PE matmul vs precomputed integer weights, scaled 1/144 on PSUM evict
 - PE transpose -> (1024,4,4) fp32 outputs
"""
import sys
sys.path.insert(0, "/opt/trn_rl_repo")
import math
import numpy as np

N_CORES = 8
BPC = 16           # batch per core
CH = 1024
KDIM = 108
HW = 729
NCHUNK = 6
TAU = 256.5
Z0 = -0.6744897501960817
DELTA = 0.30
N_ITER = 8         # 1 init probe + 7 Illinois regula-falsi passes
SHIFT = 128.0   # makes all activations positive (|t| << 128) so the
                # masked-max finisher can use is_le*t; cancels in sigmoid
SQRT_MAGIC = 0x1FBD1DF5

_cache = {}


def _phi(z):
    return 0.5 * (1.0 + math.erf(z / math.sqrt(2.0)))


def _host_consts(kernel_w, bias):
    kmat = np.ascontiguousarray(kernel_w.reshape(CH, KDIM)).astype(np.float32)
    b = np.ascontiguousarray(bias.reshape(CH)).astype(np.float32)
    kt = np.zeros((KDIM + 1, 2 * CH), np.float32)
    kt[1:, :CH] = -kmat.T
    kt[1:, CH:] = kmat.T
    kt[0, :CH] = b + SHIFT
    kt[0, CH:] = b + SHIFT
    kbar = kmat.mean(axis=0)
    mb = float(b.mean())
    siderhs = np.zeros((KDIM + 1, 2), np.float32)
    siderhs[1:, 0] = -kbar
    siderhs[1:, 1] = kbar
    siderhs[0, 0] = mb
    siderhs[0, 1] = mb
    alpha = float((kmat.astype(np.float64) ** 2).sum(axis=1).mean() / KDIM)
    bvar = float(b.astype(np.float64).var())
    a_sig = alpha
    b_sig = bvar - alpha   # sig2 = a_sig * (||p||^2 + 1) + b_sig

    # pooling matrix: integer window-multiplicity weights, scaled 1/144 later
    a_int = np.zeros((16, HW), np.float32)
    for fi in range(4):
        for fj in range(4):
            f = fi * 4 + fj
            for wi in (2 * fi, 2 * fi + 1):
                for wj in (2 * fj, 2 * fj + 1):
                    for di in range(6):
                        for dj in range(6):
                            p = (3 * wi + di) * 27 + (3 * wj + dj)
                            a_int[f, p] += 1.0
    at = np.zeros((NCHUNK * 128, 16), np.float32)
    at[:HW] = a_int.T
    at_bf16_bits = (at.view(np.uint32) >> 16).astype(np.uint16)  # exact ints
    return kt, siderhs, a_sig, b_sig, at_bf16_bits


def _build_program(n_samples, a_sig, b_sig, debug=False):
    from concourse import bacc, mybir
    import concourse.bass as bass
    from concourse.tile import TileContext
    from concourse.masks import make_identity

    f32 = mybir.dt.float32
    bf16 = mybir.dt.bfloat16
    i32 = mybir.dt.int32
    Alu = mybir.AluOpType
    Act = mybir.ActivationFunctionType

    c_lo0 = float(CH * _phi(Z0 - DELTA))
    c_hi0 = float(CH * _phi(Z0 + DELTA))

    nc = bacc.Bacc("TRN2", target_bir_lowering=False, debug=False)
    Xh = nc.dram_tensor("x", [n_samples, 3, 32, 32], f32, kind="ExternalInput")
    KTh = nc.dram_tensor("kt", [KDIM + 1, 2 * CH], f32, kind="ExternalInput")
    SRh = nc.dram_tensor("siderhs", [KDIM + 1, 2], f32, kind="ExternalInput")
    ATh = nc.dram_tensor("at", [NCHUNK * 128, 16], bf16, kind="ExternalInput")
    O1h = nc.dram_tensor("o1", [n_samples, CH, 16], f32, kind="ExternalOutput")
    O2h = nc.dram_tensor("o2", [n_samples, CH, 16], f32, kind="ExternalOutput")
    if debug:
        DTHRh = nc.dram_tensor("dthr", [n_samples, 128, 12], f32,
                               kind="ExternalOutput")
        DM0h = nc.dram_tensor("dm0", [n_samples, 128, 12], f32,
                              kind="ExternalOutput")
        DTh = nc.dram_tensor("dt0", [128, 2048], f32, kind="ExternalOutput")

    from contextlib import ExitStack
    with TileContext(nc) as tc:
        with ExitStack() as ctx:
            cpool = ctx.enter_context(tc.tile_pool(name="const", bufs=1))
            ppool = ctx.enter_context(tc.tile_pool(name="patch", bufs=3))
            p2pool = ctx.enter_context(tc.tile_pool(name="patch2", bufs=2))
            tpool = ctx.enter_context(tc.tile_pool(name="tdata", bufs=7))
            tbpool = ctx.enter_context(tc.tile_pool(name="tbdata", bufs=7))
            jpool = ctx.enter_context(tc.tile_pool(name="junk", bufs=2))
            mpool = ctx.enter_context(tc.tile_pool(name="mask", bufs=3))
            spool = ctx.enter_context(tc.tile_pool(name="state", bufs=2))
            opool = ctx.enter_context(tc.tile_pool(name="outsb", bufs=2))
            psc = ctx.enter_context(tc.tile_pool(name="psc", bufs=1, space="PSUM"))
            psp = ctx.enter_context(tc.tile_pool(name="psp", bufs=1, space="PSUM"))
            psm = ctx.enter_context(tc.tile_pool(name="psm", bufs=1, space="PSUM"))
            # ---- constants ----
            kts = cpool.tile([KDIM + 1, 2 * CH], f32)
            nc.sync.dma_start(kts[:], KTh.ap())
            srs = cpool.tile([KDIM + 1, 2], f32)
            nc.sync.dma_start(srs[:], SRh.ap())
            ones_r = cpool.tile([KDIM + 1, 1], f32)
            nc.vector.memset(ones_r[:], 1.0)
            ats = cpool.tile([128, NCHUNK * 16], bf16)
            for c in range(NCHUNK):
                nc.sync.dma_start(ats[:, 16 * c:16 * c + 16],
                                  ATh.ap()[128 * c:128 * c + 128, :])
            ident = cpool.tile([16, 16], f32)
            make_identity(nc, ident[:])
            negshift = cpool.tile([128, 1], f32)
            nc.vector.memset(negshift[:], -SHIFT)

            for s in range(n_samples):
                # ---- im2col ----
                pt = ppool.tile([KDIM + 1, NCHUNK * 128], f32, tag="pt")
                nc.gpsimd.memset(pt[:, HW:], 0.0)
                nc.vector.memset(pt[0:1, 0:HW], 1.0)
                for ci in range(3):
                    for kh in range(6):
                        k0 = 1 + ci * 36 + kh * 6
                        src = bass.AP(Xh, s * 3072 + ci * 1024 + kh * 32,
                                      [[1, 6], [32, 27], [1, 27]])
                        dst = pt[k0:k0 + 6, 0:HW].rearrange(
                            "p (a b) -> p a b", a=27)
                        nc.sync.dma_start(dst, src)
                pt2 = p2pool.tile([KDIM + 1, NCHUNK * 128], f32, tag="pt2")
                nc.vector.tensor_mul(pt2[:, 0:HW], pt[:, 0:HW], pt[:, 0:HW])
                nc.gpsimd.memset(pt2[:, HW:], 0.0)

                # ---- conv + side matmuls, evict to SBUF ----
                tsb = []
                tbb = []
                side = spool.tile([128, 4 * NCHUNK], f32, tag="side")
                for c in range(NCHUNK):
                    t_c = tpool.tile([128, 2 * CH], f32, tag="T")
                    tb_c = tbpool.tile([128, 2 * CH], bf16, tag="Tb")
                    lhs = pt[:, 128 * c:128 * c + 128]
                    for half in range(2):
                        ps = psc.tile([128, CH], f32, tag="cps")
                        for j in range(2):
                            nc.tensor.matmul(
                                ps[:, 512 * j:512 * j + 512], lhs,
                                kts[:, CH * half + 512 * j:
                                    CH * half + 512 * j + 512],
                                start=True, stop=True)
                        nc.scalar.activation(t_c[:, CH * half:CH * half + CH],
                                             ps[:], Act.Copy)
                        nc.scalar.activation(tb_c[:, CH * half:CH * half + CH],
                                             ps[:], Act.Identity,
                                             bias=negshift[:])
                    sps = psm.tile([128, 3], f32, tag="sps")
                    nc.tensor.matmul(sps[:, 0:2], lhs, srs[:],
                                     start=True, stop=True)
                    nc.tensor.matmul(sps[:, 2:3],
                                     pt2[:, 128 * c:128 * c + 128],
                                     ones_r[:], start=True, stop=True)
                    nc.scalar.activation(side[:, 4 * c:4 * c + 3], sps[:],
                                         Act.Copy)
                    tsb.append(t_c)
                    tbb.append(tb_c)

                # ---- state init ----
                sa3 = side[:].rearrange("p (c q) -> p c q", q=4)
                pn_v = sa3[:, :, 2:3].rearrange("p c q -> p (c q)")  # [128,6]
                sg2 = spool.tile([128, NCHUNK], f32, tag="sg2")
                # sig2 = a_sig * pn2p1 + b_sig  (per-chunk cols)
                nc.vector.tensor_scalar(sg2[:], pn_v, a_sig, b_sig,
                                        Alu.mult, Alu.add)
                # inv-sqrt bit trick + 1 Newton iteration -> sigma
                xb = spool.tile([128, NCHUNK], i32, tag="xb")
                nc.vector.tensor_scalar(xb[:], sg2[:].bitcast(i32), 1,
                                        None, Alu.arith_shift_right)
                nc.vector.tensor_scalar(xb[:], xb[:], SQRT_MAGIC, None,
                                        Alu.add)
                x0 = xb[:].bitcast(f32)
                rc = spool.tile([128, NCHUNK], f32, tag="rc")
                nc.vector.reciprocal(rc[:], x0)
                tn = spool.tile([128, NCHUNK], f32, tag="tn")
                nc.vector.tensor_mul(tn[:], sg2[:], rc[:])
                nc.vector.tensor_add(tn[:], tn[:], x0)
                sig = spool.tile([128, NCHUNK], f32, tag="sig")
                nc.vector.tensor_scalar_mul(sig[:], tn[:], 0.5)

                st_m = spool.tile([128, 12], f32, tag="st_m")
                st_lo = spool.tile([128, 12], f32, tag="st_lo")
                st_hi = spool.tile([128, 12], f32, tag="st_hi")
                st_clo = spool.tile([128, 12], f32, tag="st_clo")
                st_chi = spool.tile([128, 12], f32, tag="st_chi")
                st_ge = spool.tile([128, 12], f32, tag="st_ge")
                st_lge = spool.tile([128, 12], f32, tag="st_lge")
                st_geu = spool.tile([128, 12], mybir.dt.uint32, tag="st_geu")
                st_ngeu = spool.tile([128, 12], mybir.dt.uint32, tag="st_ngeu")
                st_cnt = spool.tile([128, 12], f32, tag="st_cnt")
                st_thr = spool.tile([128, 12], f32, tag="st_thr")
                sA = spool.tile([128, 12], f32, tag="sA")
                sB = spool.tile([128, 12], f32, tag="sB")
                sC = spool.tile([128, 12], f32, tag="sC")

                sig_b = sig[:].unsqueeze(2).to_broadcast([128, NCHUNK, 2])
                mu_v = sa3[:, :, 0:2]
                m_v = st_m[:].rearrange("p (c h) -> p c h", h=2)
                lo_v = st_lo[:].rearrange("p (c h) -> p c h", h=2)
                hi_v = st_hi[:].rearrange("p (c h) -> p c h", h=2)
                # m0 = mu + Z0*sigma ; lo/hi = m0 -/+ DELTA*sigma
                nc.vector.scalar_tensor_tensor(m_v, sig_b, Z0, mu_v,
                                               op0=Alu.mult, op1=Alu.add)
                nc.vector.scalar_tensor_tensor(lo_v, sig_b, -DELTA, m_v,
                                               op0=Alu.mult, op1=Alu.add)
                nc.vector.scalar_tensor_tensor(hi_v, sig_b, DELTA, m_v,
                                               op0=Alu.mult, op1=Alu.add)
                nc.vector.memset(st_clo[:], c_lo0)
                nc.vector.memset(st_chi[:], c_hi0)
                if debug:
                    nc.sync.dma_start(DM0h.ap()[s], st_m[:])

                # ---- iterations ----
                for it in range(N_ITER):
                    for c in range(4, NCHUNK):
                        for half in range(2):
                            col = 2 * c + half
                            junka = jpool.tile([128, CH], bf16, tag="junka")
                            nc.scalar.activation(
                                junka[:], tbb[c][:, CH * half:CH * half + CH],
                                Act.Sign, bias=st_m[:, col:col + 1],
                                scale=-1.0,
                                accum_out=st_cnt[:, col:col + 1])
                    for c in range(4):
                        for half in range(2):
                            col = 2 * c + half
                            junk = jpool.tile([128, CH], bf16, tag="junk")
                            nc.vector.tensor_scalar(
                                junk[:], tbb[c][:, CH * half:CH * half + CH],
                                st_m[:, col:col + 1], 0.0,
                                Alu.is_le, Alu.add,
                                accum_out=st_cnt[:, col:col + 1])
                    # ACT cols hold S = #le - #gt; cnt = 0.5*S + 512
                    nc.vector.tensor_scalar(st_cnt[:, 8:12], st_cnt[:, 8:12],
                                            0.5, 512.0, Alu.mult, Alu.add)
                    nc.vector.tensor_scalar(st_ge[:], st_cnt[:], TAU, None,
                                            Alu.is_ge)
                    nc.vector.tensor_scalar(st_geu[:], st_cnt[:], TAU, None,
                                            Alu.is_ge)
                    nc.vector.tensor_scalar(st_ngeu[:], st_cnt[:], TAU, None,
                                            Alu.is_lt)
                    if it > 0:
                        # Illinois: stale-side count pulled toward TAU
                        nc.vector.tensor_tensor(sA[:], st_ge[:], st_lge[:],
                                                op=Alu.is_equal)
                        nc.vector.tensor_mul(sB[:], sA[:], st_ge[:])  # both ge
                        nc.vector.tensor_sub(sA[:], sA[:], sB[:])     # both lt
                        nc.vector.scalar_tensor_tensor(sC[:], st_clo[:], TAU,
                                                       sB[:], op0=Alu.subtract,
                                                       op1=Alu.mult)
                        nc.vector.scalar_tensor_tensor(st_clo[:], sC[:], -0.5,
                                                       st_clo[:], op0=Alu.mult,
                                                       op1=Alu.add)
                        nc.vector.scalar_tensor_tensor(sC[:], st_chi[:], TAU,
                                                       sA[:], op0=Alu.subtract,
                                                       op1=Alu.mult)
                        nc.vector.scalar_tensor_tensor(st_chi[:], sC[:], -0.5,
                                                       st_chi[:], op0=Alu.mult,
                                                       op1=Alu.add)
                    nc.vector.copy_predicated(st_hi[:], st_geu[:], st_m[:])
                    nc.vector.copy_predicated(st_chi[:], st_geu[:], st_cnt[:])
                    nc.vector.copy_predicated(st_lo[:], st_ngeu[:], st_m[:])
                    nc.vector.copy_predicated(st_clo[:], st_ngeu[:], st_cnt[:])
                    st_ge, st_lge = st_lge, st_ge
                    if it < N_ITER - 1:
                        # regula falsi: m = lo - (c_lo-TAU)*(hi-lo)/(c_hi-c_lo)
                        nc.vector.tensor_sub(sA[:], st_hi[:], st_lo[:])
                        nc.vector.tensor_sub(sB[:], st_chi[:], st_clo[:])
                        nc.vector.reciprocal(sB[:], sB[:])
                        nc.vector.scalar_tensor_tensor(sC[:], st_clo[:], TAU,
                                                       sA[:], op0=Alu.subtract,
                                                       op1=Alu.mult)
                        nc.vector.tensor_mul(sC[:], sC[:], sB[:])
                        nc.vector.tensor_sub(st_m[:], st_lo[:], sC[:])

                # ---- exact finisher + sigmoid + pooling ----
                # fp32 threshold consistent with bf16-count space:
                # hi + |hi|*2^-9 sits strictly inside (hi, next_bf16(hi)),
                # just below the round-to-nearest boundary
                st_his = spool.tile([128, 12], f32, tag="st_his")
                nc.scalar.activation(sB[:], st_hi[:], Act.Abs,
                                     scale=1.0 / 512.0)
                nc.vector.scalar_tensor_tensor(st_his[:], sB[:], SHIFT,
                                               st_hi[:], op0=Alu.add,
                                               op1=Alu.add)
                pooled = psp.tile([16, 2 * CH], f32, tag="pool")
                for c in range(NCHUNK):
                    msk = mpool.tile([128, 2 * CH], bf16, tag="msk")
                    for half in range(2):
                        col = 2 * c + half
                        t_half = tsb[c][:, CH * half:CH * half + CH]
                        junk = jpool.tile([128, CH], f32, tag="junk")
                        nc.vector.scalar_tensor_tensor(
                            junk[:], t_half, st_his[:, col:col + 1], t_half,
                            op0=Alu.is_le, op1=Alu.mult)
                        nc.vector.tensor_reduce(st_thr[:, col:col + 1],
                                                junk[:],
                                                axis=mybir.AxisListType.X,
                                                op=Alu.max)
                        nc.scalar.activation(msk[:, CH * half:CH * half + CH],
                                             t_half, Act.Sigmoid,
                                             bias=st_thr[:, col:col + 1],
                                             scale=-1.0)
                    for j in range(4):
                        nc.tensor.matmul(pooled[:, 512 * j:512 * j + 512],
                                         ats[:, 16 * c:16 * c + 16],
                                         msk[:, 512 * j:512 * j + 512],
                                         start=(c == 0), stop=(c == NCHUNK - 1))
                if debug:
                    nc.sync.dma_start(DTHRh.ap()[s], st_thr[:])
                    if s == 0:
                        nc.sync.dma_start(DTh.ap()[:], tsb[0][:])

                # ---- scale, transpose, store ----
                pooled_sb = opool.tile([16, 2 * CH], f32, tag="pooled_sb")
                nc.scalar.activation(pooled_sb[:], pooled[:], Act.Copy,
                                     scale=1.0 / 144.0)
                pst = psm.tile([128, 256], f32, tag="pst")
                for j in range(16):
                    nc.tensor.transpose(pst[:, 16 * j:16 * j + 16],
                                        pooled_sb[:, 128 * j:128 * j + 128],
                                        ident[:])
                outt = opool.tile([128, 256], f32, tag="outt")
                nc.scalar.activation(outt[:], pst[:], Act.Copy)
                for br, Oh in ((0, O1h), (1, O2h)):
                    src = outt[:, 128 * br:128 * br + 128].rearrange(
                        "p (j f) -> p j f", f=16)
                    dst = Oh.ap()[s].rearrange("(j r) f -> r j f", r=128)
                    nc.sync.dma_start(dst, src)
    nc.compile()
    return nc


def _get_program(n_samples, a_sig, b_sig, debug=False):
    key = (n_samples, round(a_sig, 9), round(b_sig, 9), debug)
    if key not in _cache:
        _cache[key] = _build_program(n_samples, a_sig, b_sig, debug)
    return _cache[key]


def _run(x, kernel_w, bias, n_samples_per_core=BPC, debug=False, trace=False):
    from concourse.bass_utils import run_bass_kernel_spmd
    import ml_dtypes

    kt, siderhs, a_sig, b_sig, at_bits = _host_consts(kernel_w, bias)
    at_bf = at_bits.view(ml_dtypes.bfloat16)
    nc = _get_program(n_samples_per_core, a_sig, b_sig, debug)
    n_cores = x.shape[0] // n_samples_per_core
    in_maps = []
    for i in range(n_cores):
        xs = np.ascontiguousarray(
            x[i * n_samples_per_core:(i + 1) * n_samples_per_core]
        ).astype(np.float32)
        in_maps.append({"x": xs, "kt": kt, "siderhs": siderhs,
                       "at": at_bf})
    res = run_bass_kernel_spmd(nc, in_maps, list(range(n_cores)), trace=trace)
    o1 = np.concatenate([r["o1"] for r in res.results], axis=0)
    o2 = np.concatenate([r["o2"] for r in res.results], axis=0)
    B = o1.shape[0]
    out1 = o1.reshape(B, CH, 4, 4)
    out2 = o2.reshape(B, CH, 4, 4)
    return (out1, out2), res


def kernel(x, kernel, bias):
    (out1, out2), _ = _run(np.asarray(x), np.asarray(kernel),
                           np.asarray(bias))
    return out1, out2


# revision 14
# speedup vs baseline: 1.4777x; 1.2574x over previous
"""Trainium2 Bass kernel for nn_Net_34136400069284 (topk_masking).

Computes: conv2d(x, kernel, VALID) -> (B,1024,27,27); for t = -out+bias and
t = +out+bias: per-pixel 257th-smallest along channels (exact order stat),
soft mask sigmoid(thr - t), 6x6/stride-3 avg-pool, 2x2 block mean ->
(B,1024,4,4) per branch.

Strategy (per core, data-parallel over batch, 16 samples/core):
 - im2col patches [109,729] per sample (108 taps + ones row for bias fold)
 - conv as PE matmul vs augmented [109,2048] weights ([-K|+K] with bias row)
   -> per pixel-chunk [128,2048] activations T (both branches side by side)
 - per-pixel kth-smallest by bracketed counting: Gaussian-moment init
   (mean via matmul column, sigma via ||patch||^2 matmul + inv-sqrt bit trick)
   then 9 count passes (1 probe + 8 Illinois regula-falsi) using fused
   tensor_scalar(is_le)+accum; exact finisher max{t <= hi} via
   tensor_tensor_reduce
 - mask = sigmoid(thr - t) on ScalarE (bf16), both pool stages fused into one
   PE matmul vs precomputed integer weights, scaled 1/144 on PSUM evict
 - PE transpose -> (1024,4,4) fp32 outputs
"""
import sys
sys.path.insert(0, "/opt/trn_rl_repo")
import math
import numpy as np

N_CORES = 8
BPC = 16           # batch per core
CH = 1024
KDIM = 108
HW = 729
NCHUNK = 6
TAU = 256.5
Z0 = -0.6744897501960817
DELTA = 0.30
N_ITER = 7         # 1 init probe + 6 Illinois regula-falsi passes
SHIFT = 128.0   # makes all activations positive (|t| << 128) so the
                # masked-max finisher can use is_le*t; cancels in sigmoid
SQRT_MAGIC = 0x1FBD1DF5

_cache = {}


def _phi(z):
    return 0.5 * (1.0 + math.erf(z / math.sqrt(2.0)))


def _host_consts(kernel_w, bias):
    kmat = np.ascontiguousarray(kernel_w.reshape(CH, KDIM)).astype(np.float32)
    b = np.ascontiguousarray(bias.reshape(CH)).astype(np.float32)
    kt = np.zeros((KDIM + 1, 2 * CH), np.float32)
    kt[1:, :CH] = -kmat.T
    kt[1:, CH:] = kmat.T
    kt[0, :CH] = b + SHIFT
    kt[0, CH:] = b + SHIFT
    kbar = kmat.mean(axis=0)
    mb = float(b.mean())
    siderhs = np.zeros((KDIM + 1, 2), np.float32)
    siderhs[1:, 0] = -kbar
    siderhs[1:, 1] = kbar
    siderhs[0, 0] = mb + SHIFT
    siderhs[0, 1] = mb + SHIFT
    alpha = float((kmat.astype(np.float64) ** 2).sum(axis=1).mean() / KDIM)
    bvar = float(b.astype(np.float64).var())
    a_sig = alpha
    b_sig = bvar - alpha   # sig2 = a_sig * (||p||^2 + 1) + b_sig

    # pooling matrix: integer window-multiplicity weights, scaled 1/144 later
    a_int = np.zeros((16, HW), np.float32)
    for fi in range(4):
        for fj in range(4):
            f = fi * 4 + fj
            for wi in (2 * fi, 2 * fi + 1):
                for wj in (2 * fj, 2 * fj + 1):
                    for di in range(6):
                        for dj in range(6):
                            p = (3 * wi + di) * 27 + (3 * wj + dj)
                            a_int[f, p] += 1.0
    at = np.zeros((NCHUNK * 128, 16), np.float32)
    at[:HW] = a_int.T
    at_bf16_bits = (at.view(np.uint32) >> 16).astype(np.uint16)  # exact ints
    return kt, siderhs, a_sig, b_sig, at_bf16_bits


def _build_program(n_samples, a_sig, b_sig, debug=False):
    from concourse import bacc, mybir
    import concourse.bass as bass
    from concourse.tile import TileContext
    from concourse.masks import make_identity

    f32 = mybir.dt.float32
    bf16 = mybir.dt.bfloat16
    i32 = mybir.dt.int32
    Alu = mybir.AluOpType
    Act = mybir.ActivationFunctionType

    c_lo0 = float(CH * _phi(Z0 - DELTA))
    c_hi0 = float(CH * _phi(Z0 + DELTA))

    nc = bacc.Bacc("TRN2", target_bir_lowering=False, debug=False)
    Xh = nc.dram_tensor("x", [n_samples, 3, 32, 32], f32, kind="ExternalInput")
    KTh = nc.dram_tensor("kt", [KDIM + 1, 2 * CH], f32, kind="ExternalInput")
    SRh = nc.dram_tensor("siderhs", [KDIM + 1, 2], f32, kind="ExternalInput")
    ATh = nc.dram_tensor("at", [NCHUNK * 128, 16], bf16, kind="ExternalInput")
    O1h = nc.dram_tensor("o1", [n_samples, CH, 16], f32, kind="ExternalOutput")
    O2h = nc.dram_tensor("o2", [n_samples, CH, 16], f32, kind="ExternalOutput")
    if debug:
        DTHRh = nc.dram_tensor("dthr", [n_samples, 128, 12], f32,
                               kind="ExternalOutput")
        DM0h = nc.dram_tensor("dm0", [n_samples, 128, 12], f32,
                              kind="ExternalOutput")
        DTh = nc.dram_tensor("dt0", [128, 2048], f32, kind="ExternalOutput")

    from contextlib import ExitStack
    with TileContext(nc) as tc:
        with ExitStack() as ctx:
            cpool = ctx.enter_context(tc.tile_pool(name="const", bufs=1))
            ppool = ctx.enter_context(tc.tile_pool(name="patch", bufs=3))
            p2pool = ctx.enter_context(tc.tile_pool(name="patch2", bufs=2))
            tpool = ctx.enter_context(tc.tile_pool(name="tdata", bufs=8))
            jpool = ctx.enter_context(tc.tile_pool(name="junk", bufs=2))
            mpool = ctx.enter_context(tc.tile_pool(name="mask", bufs=3))
            spool = ctx.enter_context(tc.tile_pool(name="state", bufs=2))
            opool = ctx.enter_context(tc.tile_pool(name="outsb", bufs=2))
            psc = ctx.enter_context(tc.tile_pool(name="psc", bufs=1, space="PSUM"))
            psp = ctx.enter_context(tc.tile_pool(name="psp", bufs=1, space="PSUM"))
            psm = ctx.enter_context(tc.tile_pool(name="psm", bufs=1, space="PSUM"))
            # ---- constants ----
            kts = cpool.tile([KDIM + 1, 2 * CH], f32)
            nc.sync.dma_start(kts[:], KTh.ap())
            srs = cpool.tile([KDIM + 1, 2], f32)
            nc.sync.dma_start(srs[:], SRh.ap())
            ones_r = cpool.tile([KDIM + 1, 1], f32)
            nc.vector.memset(ones_r[:], 1.0)
            ats = cpool.tile([128, NCHUNK * 16], bf16)
            for c in range(NCHUNK):
                nc.sync.dma_start(ats[:, 16 * c:16 * c + 16],
                                  ATh.ap()[128 * c:128 * c + 128, :])
            ident = cpool.tile([16, 16], f32)
            make_identity(nc, ident[:])

            for s in range(n_samples):
                # ---- im2col ----
                pt = ppool.tile([KDIM + 1, NCHUNK * 128], f32, tag="pt")
                nc.gpsimd.memset(pt[:, HW:], 0.0)
                nc.vector.memset(pt[0:1, 0:HW], 1.0)
                for ci in range(3):
                    for kh in range(6):
                        k0 = 1 + ci * 36 + kh * 6
                        src = bass.AP(Xh, s * 3072 + ci * 1024 + kh * 32,
                                      [[1, 6], [32, 27], [1, 27]])
                        dst = pt[k0:k0 + 6, 0:HW].rearrange(
                            "p (a b) -> p a b", a=27)
                        nc.sync.dma_start(dst, src)
                pt2 = p2pool.tile([KDIM + 1, NCHUNK * 128], f32, tag="pt2")
                nc.vector.tensor_mul(pt2[:, 0:HW], pt[:, 0:HW], pt[:, 0:HW])
                nc.gpsimd.memset(pt2[:, HW:], 0.0)

                # ---- conv + side matmuls, evict to SBUF ----
                tsb = []
                side = spool.tile([128, 4 * NCHUNK], f32, tag="side")
                for c in range(NCHUNK):
                    t_c = tpool.tile([128, 2 * CH], f32, tag="T")
                    lhs = pt[:, 128 * c:128 * c + 128]
                    for half in range(2):
                        ps = psc.tile([128, CH], f32, tag="cps")
                        for j in range(2):
                            nc.tensor.matmul(
                                ps[:, 512 * j:512 * j + 512], lhs,
                                kts[:, CH * half + 512 * j:
                                    CH * half + 512 * j + 512],
                                start=True, stop=True)
                        nc.scalar.activation(t_c[:, CH * half:CH * half + CH],
                                             ps[:], Act.Copy)
                    sps = psm.tile([128, 3], f32, tag="sps")
                    nc.tensor.matmul(sps[:, 0:2], lhs, srs[:],
                                     start=True, stop=True)
                    nc.tensor.matmul(sps[:, 2:3],
                                     pt2[:, 128 * c:128 * c + 128],
                                     ones_r[:], start=True, stop=True)
                    nc.scalar.activation(side[:, 4 * c:4 * c + 3], sps[:],
                                         Act.Copy)
                    tsb.append(t_c)

                # ---- state init ----
                sa3 = side[:].rearrange("p (c q) -> p c q", q=4)
                pn_v = sa3[:, :, 2:3].rearrange("p c q -> p (c q)")  # [128,6]
                sg2 = spool.tile([128, NCHUNK], f32, tag="sg2")
                # sig2 = a_sig * pn2p1 + b_sig  (per-chunk cols)
                nc.vector.tensor_scalar(sg2[:], pn_v, a_sig, b_sig,
                                        Alu.mult, Alu.add)
                # inv-sqrt bit trick + 1 Newton iteration -> sigma
                xb = spool.tile([128, NCHUNK], i32, tag="xb")
                nc.vector.tensor_scalar(xb[:], sg2[:].bitcast(i32), 1,
                                        None, Alu.arith_shift_right)
                nc.vector.tensor_scalar(xb[:], xb[:], SQRT_MAGIC, None,
                                        Alu.add)
                x0 = xb[:].bitcast(f32)
                rc = spool.tile([128, NCHUNK], f32, tag="rc")
                nc.vector.reciprocal(rc[:], x0)
                tn = spool.tile([128, NCHUNK], f32, tag="tn")
                nc.vector.tensor_mul(tn[:], sg2[:], rc[:])
                nc.vector.tensor_add(tn[:], tn[:], x0)
                sig = spool.tile([128, NCHUNK], f32, tag="sig")
                nc.vector.tensor_scalar_mul(sig[:], tn[:], 0.5)

                st_m = spool.tile([128, 12], f32, tag="st_m")
                st_lo = spool.tile([128, 12], f32, tag="st_lo")
                st_hi = spool.tile([128, 12], f32, tag="st_hi")
                st_clo = spool.tile([128, 12], f32, tag="st_clo")
                st_chi = spool.tile([128, 12], f32, tag="st_chi")
                st_ge = spool.tile([128, 12], f32, tag="st_ge")
                st_lge = spool.tile([128, 12], f32, tag="st_lge")
                st_geu = spool.tile([128, 12], mybir.dt.uint32, tag="st_geu")
                st_ngeu = spool.tile([128, 12], mybir.dt.uint32, tag="st_ngeu")
                st_cnt = spool.tile([128, 12], f32, tag="st_cnt")
                st_thr = spool.tile([128, 12], f32, tag="st_thr")
                sA = spool.tile([128, 12], f32, tag="sA")
                sB = spool.tile([128, 12], f32, tag="sB")
                sC = spool.tile([128, 12], f32, tag="sC")

                sig_b = sig[:].unsqueeze(2).to_broadcast([128, NCHUNK, 2])
                mu_v = sa3[:, :, 0:2]
                m_v = st_m[:].rearrange("p (c h) -> p c h", h=2)
                lo_v = st_lo[:].rearrange("p (c h) -> p c h", h=2)
                hi_v = st_hi[:].rearrange("p (c h) -> p c h", h=2)
                # m0 = mu + Z0*sigma ; lo/hi = m0 -/+ DELTA*sigma
                nc.vector.scalar_tensor_tensor(m_v, sig_b, Z0, mu_v,
                                               op0=Alu.mult, op1=Alu.add)
                nc.vector.scalar_tensor_tensor(lo_v, sig_b, -DELTA, m_v,
                                               op0=Alu.mult, op1=Alu.add)
                nc.vector.scalar_tensor_tensor(hi_v, sig_b, DELTA, m_v,
                                               op0=Alu.mult, op1=Alu.add)
                nc.vector.memset(st_clo[:], c_lo0)
                nc.vector.memset(st_chi[:], c_hi0)
                if debug:
                    nc.sync.dma_start(DM0h.ap()[s], st_m[:])

                # ---- iterations ----
                ACT_C0 = 2   # chunks >= ACT_C0 counted on ScalarE
                for it in range(N_ITER):
                    for c in range(ACT_C0, NCHUNK):
                        for half in range(2):
                            col = 2 * c + half
                            junka = jpool.tile([128, CH], bf16, tag="junka")
                            nc.scalar.activation(
                                junka[:], tsb[c][:, CH * half:CH * half + CH],
                                Act.Sign, bias=st_m[:, col:col + 1],
                                scale=-1.0,
                                accum_out=st_cnt[:, col:col + 1])
                    for c in range(ACT_C0):
                        for half in range(2):
                            col = 2 * c + half
                            junk = jpool.tile([128, CH], f32, tag="junk")
                            nc.vector.tensor_scalar(
                                junk[:], tsb[c][:, CH * half:CH * half + CH],
                                st_m[:, col:col + 1], 0.0,
                                Alu.is_le, Alu.add,
                                accum_out=st_cnt[:, col:col + 1])
                    # ACT cols hold S = #le - #gt; cnt = 0.5*S + 512
                    nc.vector.tensor_scalar(st_cnt[:, 2 * ACT_C0:12],
                                            st_cnt[:, 2 * ACT_C0:12],
                                            0.5, 512.0, Alu.mult, Alu.add)
                    nc.vector.tensor_scalar(st_ge[:], st_cnt[:], TAU, None,
                                            Alu.is_ge)
                    nc.vector.tensor_scalar(st_geu[:], st_cnt[:], TAU, None,
                                            Alu.is_ge)
                    nc.vector.tensor_scalar(st_ngeu[:], st_cnt[:], TAU, None,
                                            Alu.is_lt)
                    if it > 0:
                        # Illinois: stale-side count pulled toward TAU
                        nc.vector.tensor_tensor(sA[:], st_ge[:], st_lge[:],
                                                op=Alu.is_equal)
                        nc.vector.tensor_mul(sB[:], sA[:], st_ge[:])  # both ge
                        nc.vector.tensor_sub(sA[:], sA[:], sB[:])     # both lt
                        nc.vector.scalar_tensor_tensor(sC[:], st_clo[:], TAU,
                                                       sB[:], op0=Alu.subtract,
                                                       op1=Alu.mult)
                        nc.vector.scalar_tensor_tensor(st_clo[:], sC[:], -0.5,
                                                       st_clo[:], op0=Alu.mult,
                                                       op1=Alu.add)
                        nc.vector.scalar_tensor_tensor(sC[:], st_chi[:], TAU,
                                                       sA[:], op0=Alu.subtract,
                                                       op1=Alu.mult)
                        nc.vector.scalar_tensor_tensor(st_chi[:], sC[:], -0.5,
                                                       st_chi[:], op0=Alu.mult,
                                                       op1=Alu.add)
                    nc.vector.copy_predicated(st_hi[:], st_geu[:], st_m[:])
                    nc.vector.copy_predicated(st_chi[:], st_geu[:], st_cnt[:])
                    nc.vector.copy_predicated(st_lo[:], st_ngeu[:], st_m[:])
                    nc.vector.copy_predicated(st_clo[:], st_ngeu[:], st_cnt[:])
                    st_ge, st_lge = st_lge, st_ge
                    if it < N_ITER - 1:
                        # regula falsi: m = lo - (c_lo-TAU)*(hi-lo)/(c_hi-c_lo)
                        nc.vector.tensor_sub(sA[:], st_hi[:], st_lo[:])
                        nc.vector.tensor_sub(sB[:], st_chi[:], st_clo[:])
                        nc.vector.reciprocal(sB[:], sB[:])
                        nc.vector.scalar_tensor_tensor(sC[:], st_clo[:], TAU,
                                                       sA[:], op0=Alu.subtract,
                                                       op1=Alu.mult)
                        nc.vector.tensor_mul(sC[:], sC[:], sB[:])
                        nc.vector.tensor_sub(st_m[:], st_lo[:], sC[:])

                # ---- exact finisher + sigmoid + pooling ----
                pooled = psp.tile([16, 2 * CH], f32, tag="pool")
                for c in range(NCHUNK):
                    msk = mpool.tile([128, 2 * CH], bf16, tag="msk")
                    for half in range(2):
                        col = 2 * c + half
                        t_half = tsb[c][:, CH * half:CH * half + CH]
                        junk = jpool.tile([128, CH], f32, tag="junk")
                        nc.vector.scalar_tensor_tensor(
                            junk[:], t_half, st_hi[:, col:col + 1], t_half,
                            op0=Alu.is_le, op1=Alu.mult)
                        nc.vector.tensor_reduce(st_thr[:, col:col + 1],
                                                junk[:],
                                                axis=mybir.AxisListType.X,
                                                op=Alu.max)
                        nc.scalar.activation(msk[:, CH * half:CH * half + CH],
                                             t_half, Act.Sigmoid,
                                             bias=st_thr[:, col:col + 1],
                                             scale=-1.0)
                    for j in range(4):
                        nc.tensor.matmul(pooled[:, 512 * j:512 * j + 512],
                                         ats[:, 16 * c:16 * c + 16],
                                         msk[:, 512 * j:512 * j + 512],
                                         start=(c == 0), stop=(c == NCHUNK - 1))
                if debug:
                    nc.sync.dma_start(DTHRh.ap()[s], st_thr[:])
                    if s == 0:
                        nc.sync.dma_start(DTh.ap()[:], tsb[0][:])

                # ---- scale, transpose, store ----
                pooled_sb = opool.tile([16, 2 * CH], f32, tag="pooled_sb")
                nc.scalar.activation(pooled_sb[:], pooled[:], Act.Copy,
                                     scale=1.0 / 144.0)
                pst = psm.tile([128, 256], f32, tag="pst")
                for j in range(16):
                    nc.tensor.transpose(pst[:, 16 * j:16 * j + 16],
                                        pooled_sb[:, 128 * j:128 * j + 128],
                                        ident[:])
                outt = opool.tile([128, 256], f32, tag="outt")
                nc.scalar.activation(outt[:], pst[:], Act.Copy)
                for br, Oh in ((0, O1h), (1, O2h)):
                    src = outt[:, 128 * br:128 * br + 128].rearrange(
                        "p (j f) -> p j f", f=16)
                    dst = Oh.ap()[s].rearrange("(j r) f -> r j f", r=128)
                    nc.sync.dma_start(dst, src)
    nc.compile()
    return nc


def _get_program(n_samples, a_sig, b_sig, debug=False):
    key = (n_samples, round(a_sig, 9), round(b_sig, 9), debug)
    if key not in _cache:
        _cache[key] = _build_program(n_samples, a_sig, b_sig, debug)
    return _cache[key]


def _run(x, kernel_w, bias, n_samples_per_core=BPC, debug=False, trace=False):
    from concourse.bass_utils import run_bass_kernel_spmd
    import ml_dtypes

    kt, siderhs, a_sig, b_sig, at_bits = _host_consts(kernel_w, bias)
    at_bf = at_bits.view(ml_dtypes.bfloat16)
    nc = _get_program(n_samples_per_core, a_sig, b_sig, debug)
    n_cores = x.shape[0] // n_samples_per_core
    in_maps = []
    for i in range(n_cores):
        xs = np.ascontiguousarray(
            x[i * n_samples_per_core:(i + 1) * n_samples_per_core]
        ).astype(np.float32)
        in_maps.append({"x": xs, "kt": kt, "siderhs": siderhs,
                       "at": at_bf})
    res = run_bass_kernel_spmd(nc, in_maps, list(range(n_cores)), trace=trace)
    o1 = np.concatenate([r["o1"] for r in res.results], axis=0)
    o2 = np.concatenate([r["o2"] for r in res.results], axis=0)
    B = o1.shape[0]
    out1 = o1.reshape(B, CH, 4, 4)
    out2 = o2.reshape(B, CH, 4, 4)
    return (out1, out2), res


def kernel(x, kernel, bias):
    (out1, out2), _ = _run(np.asarray(x), np.asarray(kernel),
                           np.asarray(bias))
    return out1, out2
